# revision 6
# baseline (speedup 1.0000x reference)
"""Trainium2 Bass kernel for CausalDownsamplingLRU.

Algorithm (per core = one batch element; 8 cores, data-parallel over batch):
  With lam = r*e^{i theta} (per state n), h_t = lam*h_{t-1} + Bu_t, and only
  y[:, -DS:] needed:

  1. Input GEMMs (fp16, PE): Bu^T[n,t] = (gamma*B)^T.T @ x^T  (re & im planes)
  2. FIRST half: only h_{1023} (the carry into the output window) is needed.
     Truncated to the last WCON=512 steps (error <= r^512 <= 0.99^512 ~ 0.6%):
        w_re[n,i] = sum_s V_re[s,n] x_s[i],  V = lam^{511-s}   (PE GEMMs)
        a[n] = sum_i (gamma*B)[n,i] (.) w[n,i]                 (DVE STT reduce)
  3. SECOND half: phase twist e_j = e^{-i j theta} (.) Bu_{1024+j} decouples
     the complex recurrence into two REAL per-partition scans
        s_j = r*s_{j-1} + e_j   (tensor_tensor_scan, fp32 state),
     with initial s_{-1} = e^{i theta} * h_1023.
  4. Untwist h = e^{+i j theta} (.) s, then output GEMMs:
        y^T = C_re^T.T @ h_re + (-C_im^T).T @ h_im + D^T.T @ x^T

Scheduling: DMAs are chunked and priority-ordered so the PE starts ~2us in;
W-GEMMs interleave with input GEMMs per n-block so carries are ready as the
twists finish; D@x output matmuls fill the PE gap while the scans run; output
C-matmuls accumulate per n-block as untwists land.
"""
import numpy as np

import concourse.bass as bass
import concourse.bacc as bacc
import concourse.mybir as mybir
from concourse.tile import TileContext
from concourse.bass_utils import run_bass_kernel_spmd

BATCH, T, IN, OUT, N = 8, 2048, 512, 512, 512
DS = 1024
P = 128
NB = N // P    # 4 state blocks
IBN = IN // P  # 4 input blocks
OBN = OUT // P # 4 output blocks
HF = 1024      # window length (= DS)
HH = 512       # output-gemm moving free-dim (PSUM bank limit for f32 out)
WCON = 512     # carry W-GEMM contraction (last WCON steps of first half)
WB = WCON // P

f32 = mybir.dt.float32
f16 = mybir.dt.float16
AOP = mybir.AluOpType

GP_STT = False   # offload carry row-dot reduces to gpsimd
GP_UNTW = False  # offload one untwist partner product per (nb, h) to gpsimd

_CACHE = {}


def _build_nc():
    if "nc" in _CACHE:
        return _CACHE["nc"]
    nc = bacc.Bacc()
    xT = nc.dram_tensor("xT", [IN, HF], f16, kind="ExternalInput")    # 2nd half, [i, t]
    xw = nc.dram_tensor("xw", [WCON, IN], f16, kind="ExternalInput")  # x[512:1024], [s, i]
    btr = nc.dram_tensor("btr", [IN, N], f16, kind="ExternalInput")
    bti = nc.dram_tensor("bti", [IN, N], f16, kind="ExternalInput")
    vre = nc.dram_tensor("vre", [WCON, N], f16, kind="ExternalInput")
    vim = nc.dram_tensor("vim", [WCON, N], f16, kind="ExternalInput")
    bn2 = nc.dram_tensor("bn2", [N, 2 * IN], f16, kind="ExternalInput")
    bn3 = nc.dram_tensor("bn3", [N, 2 * IN], f16, kind="ExternalInput")
    cosj = nc.dram_tensor("cosj", [N, HF], f16, kind="ExternalInput")
    sinj = nc.dram_tensor("sinj", [N, HF], f16, kind="ExternalInput")
    rb = nc.dram_tensor("rb", [N, 1], f32, kind="ExternalInput")
    # rot columns: 0=cos(theta), 1=-sin(theta), 2=sin(theta)
    rot = nc.dram_tensor("rot", [N, 3], f32, kind="ExternalInput")
    ctr = nc.dram_tensor("ctr", [N, OUT], f16, kind="ExternalInput")
    ctin = nc.dram_tensor("ctin", [N, OUT], f16, kind="ExternalInput")
    dtw = nc.dram_tensor("dtw", [IN, OUT], f16, kind="ExternalInput")
    yT = nc.dram_tensor("yT", [OUT, DS], f16, kind="ExternalOutput")

    with TileContext(nc) as tc:
        with (
            tc.tile_pool(name="const", bufs=1) as cp,
            tc.tile_pool(name="work", bufs=1) as wkp,
            tc.tile_pool(name="ps", bufs=3, space="PSUM") as bp,
            tc.tile_pool(name="yps", bufs=5, space="PSUM") as yp,
        ):
            # ---------------- DMA loads (priority-chunked) ----------------
            def row_tiles(dram, eng, pfx, rows=None, col0=0, cols=None):
                """Load dram[rb*P:(rb+1)*P, col0:col0+cols] per row-block."""
                rows = rows if rows is not None else dram.shape[0]
                cols = cols if cols is not None else dram.shape[1]
                out = []
                for i in range(rows // P):
                    t = cp.tile([P, cols], dram.dtype, tag=f"{pfx}{i}", name=f"{pfx}{i}")
                    eng.dma_start(t[:], dram[i * P:(i + 1) * P, col0:col0 + cols])
                    out.append(t)
                return out

            # scalar queue: input weights, then output weights
            btr_t = row_tiles(btr, nc.scalar, "btr")
            bti_t = row_tiles(bti, nc.scalar, "bti")
            # sync queue: x second half (h0 halves first), then bn tables
            xts = []
            for i in range(IBN):
                t = cp.tile([P, HF], f16, tag=f"xt{i}", name=f"xt{i}")
                nc.sync.dma_start(t[:, 0:HH], xT[i * P:(i + 1) * P, 0:HH])
                xts.append(t)
            for i in range(IBN):
                nc.sync.dma_start(xts[i][:, HH:HF], xT[i * P:(i + 1) * P, HH:HF])
            # gpsimd queue: w-gemm operands, twist tables, small consts
            xw_t = row_tiles(xw, nc.gpsimd, "xw")
            vre_t = row_tiles(vre, nc.gpsimd, "vre")
            vim_t = row_tiles(vim, nc.gpsimd, "vim")
            cos_t = [None] * NB
            sin_t = [None] * NB
            def load_trig(nb):
                c = cp.tile([P, HF], f16, tag=f"cos{nb}", name=f"cos{nb}")
                s = cp.tile([P, HF], f16, tag=f"sin{nb}", name=f"sin{nb}")
                nc.gpsimd.dma_start(c[:], cosj[nb * P:(nb + 1) * P, :])
                nc.gpsimd.dma_start(s[:], sinj[nb * P:(nb + 1) * P, :])
                cos_t[nb], sin_t[nb] = c, s
            load_trig(0)
            bn2_t = row_tiles(bn2, nc.sync, "bn2")
            load_trig(1)
            rb_t = row_tiles(rb, nc.gpsimd, "rb")
            rot_t = row_tiles(rot, nc.gpsimd, "rot")
            bn3_t = row_tiles(bn3, nc.sync, "bn3")
            load_trig(2)
            load_trig(3)
            # output weights: scalar queue (idle after bti)
            dtw_t = row_tiles(dtw, nc.scalar, "dtw")
            ctr_t = row_tiles(ctr, nc.scalar, "ctr")
            ctin_t = row_tiles(ctin, nc.scalar, "ctin")

            # ---------------- PE phase 1: input + W GEMMs ----------------
            bus = {}    # (nb, plane) -> [P, HF] f16 Bu tile
            w2s = [None] * NB  # [P, 2*IN] f16 [w_re | w_im]

            def input_gemm(nb):
                nsl = slice(nb * P, (nb + 1) * P)
                for pi, wtiles in enumerate((btr_t, bti_t)):
                    bu = wkp.tile([P, HF], f16, tag=f"bu{pi}", bufs=2,
                                  name=f"bu{pi}_{nb}")
                    ps = [bp.tile([P, HH], f32, tag="ps", name=f"bups{h}")
                          for h in range(2)]
                    for ib in range(IBN):
                        for h in range(2):  # same stationary weight, 2 moving
                            nc.tensor.matmul(
                                ps[h][:], wtiles[ib][:, nsl],
                                xts[ib][:, h * HH:(h + 1) * HH],
                                start=(ib == 0), stop=(ib == IBN - 1))
                    for h in range(2):
                        nc.scalar.copy(bu[:, h * HH:(h + 1) * HH], ps[h][:])
                    bus[(nb, pi)] = bu

            def w_gemm(nb):
                nsl = slice(nb * P, (nb + 1) * P)
                w2 = wkp.tile([P, 2 * IN], f16, tag="w2", bufs=2, name=f"w2_{nb}")
                for pi, vt in enumerate((vre_t, vim_t)):
                    ps = bp.tile([P, IN], f32, tag="ps", name="wps")
                    for sb in range(WB):
                        nc.tensor.matmul(ps[:], vt[sb][:, nsl], xw_t[sb][:],
                                         start=(sb == 0), stop=(sb == WB - 1))
                    nc.scalar.copy(w2[:, pi * IN:(pi + 1) * IN], ps[:])
                w2s[nb] = w2

            # ---------------- DVE helpers ----------------
            dump = wkp.tile([P, 2 * IN], f16, tag="dump", name="dump")
            dump2 = wkp.tile([P, 2 * IN], f16, tag="dump2", name="dump2")
            carries = {}

            def carry_reduce(nb):
                """a = sum_i bn (.) w2 (row-dot), then init = e^{i theta}*a."""
                a_re = cp.tile([P, 1], f32, tag=f"are{nb}", name=f"are{nb}")
                a_im = cp.tile([P, 1], f32, tag=f"aim{nb}", name=f"aim{nb}")
                eng_im = nc.gpsimd if GP_STT else nc.vector
                nc.vector.scalar_tensor_tensor(
                    dump[:], bn2_t[nb][:], 1.0, w2s[nb][:], AOP.bypass, AOP.mult,
                    accum_out=a_re[:])
                eng_im.scalar_tensor_tensor(
                    dump2[:], bn3_t[nb][:], 1.0, w2s[nb][:], AOP.bypass, AOP.mult,
                    accum_out=a_im[:])
                carries[nb] = (a_re, a_im)

            inits = {}

            def carry_rot(nb):
                a_re, a_im = carries[nb]
                i_re = cp.tile([P, 1], f32, tag=f"ire{nb}", name=f"ire{nb}")
                i_im = cp.tile([P, 1], f32, tag=f"iim{nb}", name=f"iim{nb}")
                u_re = cp.tile([P, 1], f32, tag=f"ure{nb}", name=f"ure{nb}")
                u_im = cp.tile([P, 1], f32, tag=f"uim{nb}", name=f"uim{nb}")
                nc.scalar.mul(u_re[:], a_re[:], rot_t[nb][:, 0:1])
                nc.vector.scalar_tensor_tensor(
                    i_re[:], a_im[:], rot_t[nb][:, 1:2], u_re[:],
                    AOP.mult, AOP.add)
                nc.scalar.mul(u_im[:], a_im[:], rot_t[nb][:, 0:1])
                nc.vector.scalar_tensor_tensor(
                    i_im[:], a_re[:], rot_t[nb][:, 2:3], u_im[:],
                    AOP.mult, AOP.add)
                inits[nb] = (i_re, i_im)

            es = {}

            def twist(nb):
                ct, st = cos_t[nb], sin_t[nb]
                br, bi = bus[(nb, 0)], bus[(nb, 1)]
                p1 = wkp.tile([P, HF], f16, tag="p1", bufs=2, name="p1")
                p2 = wkp.tile([P, HF], f16, tag="p2", bufs=2, name="p2")
                e_re = wkp.tile([P, HF], f16, tag="er", bufs=4, name=f"er{nb}")
                e_im = wkp.tile([P, HF], f16, tag="ei", bufs=4, name=f"ei{nb}")
                nc.vector.tensor_tensor(p1[:], ct[:], br[:], AOP.mult)
                nc.vector.tensor_tensor(p2[:], st[:], bi[:], AOP.mult)
                nc.vector.tensor_tensor(e_re[:], p1[:], p2[:], AOP.add)
                nc.vector.tensor_tensor(p1[:], ct[:], bi[:], AOP.mult)
                nc.vector.tensor_tensor(p2[:], st[:], br[:], AOP.mult)
                nc.vector.tensor_tensor(e_im[:], p1[:], p2[:], AOP.subtract)
                es[nb] = (e_re, e_im)

            ss = {}

            def scan(nb):
                i_re, i_im = inits[nb]
                e_re, e_im = es[nb]
                s_re = wkp.tile([P, HF], f16, tag="sr", bufs=4, name=f"sr{nb}")
                s_im = wkp.tile([P, HF], f16, tag="si", bufs=4, name=f"si{nb}")
                rbb = rb_t[nb][:, 0:1].broadcast_to((P, HF))
                nc.vector.tensor_tensor_scan(
                    s_re[:], rbb, e_re[:], i_re[:, 0:1], AOP.mult, AOP.add)
                nc.vector.tensor_tensor_scan(
                    s_im[:], rbb, e_im[:], i_im[:, 0:1], AOP.mult, AOP.add)
                ss[nb] = (s_re, s_im)

            hhs = {}

            def untwist(nb, h):
                hs = slice(h * HH, (h + 1) * HH)
                s_re, s_im = ss[nb]
                ct, st = cos_t[nb], sin_t[nb]
                if h == 0:
                    hhr = wkp.tile([P, HF], f16, tag="hhr", bufs=4, name=f"hhr{nb}")
                    hhi = wkp.tile([P, HF], f16, tag="hhi", bufs=4, name=f"hhi{nb}")
                    hhs[nb] = (hhr, hhi)
                hhr, hhi = hhs[nb]
                q1 = wkp.tile([P, HH], f16, tag="q1", bufs=2, name="q1")
                q2 = wkp.tile([P, HH], f16, tag="q2", bufs=2, name="q2")
                q3 = wkp.tile([P, HH], f16, tag="q3", bufs=2, name="q3")
                q4 = wkp.tile([P, HH], f16, tag="q4", bufs=2, name="q4")
                eng4 = nc.gpsimd if GP_UNTW else nc.vector
                eng4.tensor_tensor(q4[:], st[:, hs], s_re[:, hs], AOP.mult)
                nc.vector.tensor_tensor(q1[:], ct[:, hs], s_re[:, hs], AOP.mult)
                nc.vector.tensor_tensor(q2[:], st[:, hs], s_im[:, hs], AOP.mult)
                nc.vector.tensor_tensor(hhr[:, hs], q1[:], q2[:], AOP.subtract)
                nc.vector.tensor_tensor(q3[:], ct[:, hs], s_im[:, hs], AOP.mult)
                nc.vector.tensor_tensor(hhi[:, hs], q3[:], q4[:], AOP.add)

            # ---------------- emit phase 1 + DVE chain ----------------
            input_gemm(0)
            w_gemm(0)
            twist(0)
            carry_reduce(0)
            input_gemm(1)
            w_gemm(1)
            twist(1)
            carry_reduce(1)
            input_gemm(2)
            w_gemm(2)
            twist(2)
            carry_reduce(2)
            input_gemm(3)
            w_gemm(3)
            twist(3)
            carry_reduce(3)
            for nb in range(NB):
                carry_rot(nb)

            # ---------------- output GEMMs: D@x first (fills PE gap) -----
            NMM = 2 * NB + IBN  # accumulation ops per output group
            groups = {}
            for h in range(2):
                for ob in range(OBN):
                    gi = h * OBN + ob
                    pool = yp if gi < 5 else bp
                    tag = "yps" if gi < 5 else "ps"
                    groups[(h, ob)] = pool.tile([P, HH], f32, tag=tag, name="yps")
            # dtw matmuls: weight-stationary over (h in groups)
            for ob in range(OBN):
                osl = slice(ob * P, (ob + 1) * P)
                for ib in range(IBN):
                    for h in range(2):
                        nc.tensor.matmul(
                            groups[(h, ob)][:], dtw_t[ib][:, osl],
                            xts[ib][:, h * HH:(h + 1) * HH],
                            start=(ib == 0), stop=False)

            # ---------------- scans + untwists + C matmuls ----------------
            def c_mms(nb):
                hhr, hhi = hhs[nb]
                last = nb == NB - 1
                for ob in range(OBN):
                    osl = slice(ob * P, (ob + 1) * P)
                    for wi, (wt, m) in enumerate(
                            ((ctr_t[nb][:, osl], hhr), (ctin_t[nb][:, osl], hhi))):
                        for h in range(2):
                            nc.tensor.matmul(
                                groups[(h, ob)][:], wt, m[:, h * HH:(h + 1) * HH],
                                start=False, stop=(last and wi == 1))

            for nb in range(NB):
                scan(nb)
                untwist(nb, 0)
                untwist(nb, 1)
                c_mms(nb)

            # ---------------- evac + store ----------------
            for h in range(2):
                for ob in range(OBN):
                    osl = slice(ob * P, (ob + 1) * P)
                    hsl = slice(h * HH, (h + 1) * HH)
                    ysb = wkp.tile([P, HH], f16, tag="ysb", bufs=2, name="ysb")
                    nc.scalar.copy(ysb[:], groups[(h, ob)][:])
                    nc.sync.dma_start(yT[osl, hsl], ysb[:])

    nc.compile()
    nc.finalize()
    _CACHE["nc"] = nc
    return nc


def _host_prep(x, nu_log, theta_log, gamma_log, B_re, B_im, C_re, C_im, D):
    f64 = np.float64
    nu = np.asarray(nu_log, f64)
    th = np.asarray(theta_log, f64)
    gl = np.asarray(gamma_log, f64)
    r = np.exp(-np.exp(nu))
    theta = np.exp(th)
    gamma = np.exp(gl)

    gbr = gamma[:, None] * np.asarray(B_re, f64)
    gbi = gamma[:, None] * np.asarray(B_im, f64)
    shared = {
        "btr": np.ascontiguousarray(gbr.T).astype(np.float16),
        "bti": np.ascontiguousarray(gbi.T).astype(np.float16),
        "ctr": np.ascontiguousarray(np.asarray(C_re, f64).T).astype(np.float16),
        "ctin": np.ascontiguousarray((-np.asarray(C_im, f64)).T).astype(np.float16),
        "dtw": np.ascontiguousarray(np.asarray(D, f64).T).astype(np.float16),
    }
    j = np.arange(HF, dtype=f64)
    ang = theta[:, None] * j[None, :]
    shared["cosj"] = np.cos(ang).astype(np.float16)
    shared["sinj"] = np.sin(ang).astype(np.float16)
    # V = lam^{511-s} over the LAST WCON steps of the first half, [s, n]
    e = (WCON - 1) - np.arange(WCON, dtype=f64)
    mag = np.exp(np.log(r)[:, None] * e[None, :])
    angv = theta[:, None] * e[None, :]
    shared["vre"] = np.ascontiguousarray((mag * np.cos(angv)).T).astype(np.float16)
    shared["vim"] = np.ascontiguousarray((mag * np.sin(angv)).T).astype(np.float16)
    shared["bn2"] = np.concatenate([gbr, -gbi], axis=1).astype(np.float16)
    shared["bn3"] = np.concatenate([gbi, gbr], axis=1).astype(np.float16)
    shared["rb"] = np.ascontiguousarray(r[:, None].astype(np.float32))
    shared["rot"] = np.stack(
        [np.cos(theta), -np.sin(theta), np.sin(theta)], axis=1).astype(np.float32)

    x = np.asarray(x, np.float32)
    in_maps = []
    for b in range(BATCH):
        m = dict(shared)
        m["xT"] = np.ascontiguousarray(x[b, HF:].T).astype(np.float16)
        m["xw"] = np.ascontiguousarray(x[b, HF - WCON:HF]).astype(np.float16)
        in_maps.append(m)
    return in_maps


def _run(in_maps, trace=False):
    nc = _build_nc()
    return run_bass_kernel_spmd(nc, in_maps, core_ids=list(range(BATCH)), trace=trace)


def kernel(**inputs):
    in_maps = _host_prep(**inputs)
    res = _run(in_maps, trace=False)
    y = np.stack([np.ascontiguousarray(res.results[b]["yT"].T) for b in range(BATCH)])
    return y.astype(np.float32)


def kernel_traced(**inputs):
    """Like kernel() but returns (y, exec_time_ns). Used by test.py."""
    in_maps = _host_prep(**inputs)
    res = _run(in_maps, trace=True)
    y = np.stack([np.ascontiguousarray(res.results[b]["yT"].T) for b in range(BATCH)])
    return y.astype(np.float32), res.exec_time_ns


# revision 9
# speedup vs baseline: 1.0177x; 1.0177x over previous
"""Trainium2 Bass kernel for CausalDownsamplingLRU.

Algorithm (per core = one batch element; 8 cores, data-parallel over batch):
  With lam = r*e^{i theta} (per state n), h_t = lam*h_{t-1} + Bu_t, and only
  y[:, -DS:] needed:

  1. Input GEMMs (fp16, PE): Bu^T[n,t] = (gamma*B)^T.T @ x^T  (re & im planes)
  2. FIRST half: only h_{1023} (the carry into the output window) is needed.
     Truncated to the last WCON=512 steps (error <= r^512 <= 0.99^512 ~ 0.6%):
        w_re[n,i] = sum_s V_re[s,n] x_s[i],  V = lam^{511-s}   (PE GEMMs)
        a[n] = sum_i (gamma*B)[n,i] (.) w[n,i]                 (DVE STT reduce)
  3. SECOND half: phase twist e_j = e^{-i j theta} (.) Bu_{1024+j} decouples
     the complex recurrence into two REAL per-partition scans
        s_j = r*s_{j-1} + e_j   (tensor_tensor_scan, fp32 state),
     with initial s_{-1} = e^{i theta} * h_1023.
  4. Untwist h = e^{+i j theta} (.) s, then output GEMMs:
        y^T = C_re^T.T @ h_re + (-C_im^T).T @ h_im + D^T.T @ x^T

Scheduling: DMAs are chunked and priority-ordered so the PE starts ~2us in;
W-GEMMs interleave with input GEMMs per n-block so carries are ready as the
twists finish; D@x output matmuls fill the PE gap while the scans run; output
C-matmuls accumulate per n-block as untwists land.
"""
import numpy as np

import concourse.bass as bass
import concourse.bacc as bacc
import concourse.mybir as mybir
from concourse.tile import TileContext
from concourse.bass_utils import run_bass_kernel_spmd

BATCH, T, IN, OUT, N = 8, 2048, 512, 512, 512
DS = 1024
P = 128
NB = N // P    # 4 state blocks
IBN = IN // P  # 4 input blocks
OBN = OUT // P # 4 output blocks
HF = 1024      # window length (= DS)
HH = 512       # output-gemm moving free-dim (PSUM bank limit for f32 out)
WCON = 512     # carry W-GEMM contraction (last WCON steps of first half)
WB = WCON // P

f32 = mybir.dt.float32
f16 = mybir.dt.float16
AOP = mybir.AluOpType

GP_STT = False   # offload carry row-dot reduces to gpsimd
GP_UNTW = False  # offload one untwist partner product per (nb, h) to gpsimd

_CACHE = {}


def _build_nc():
    if "nc" in _CACHE:
        return _CACHE["nc"]
    nc = bacc.Bacc()
    xT = nc.dram_tensor("xT", [IN, HF], f16, kind="ExternalInput")    # 2nd half, [i, t]
    xw = nc.dram_tensor("xw", [WCON, IN], f16, kind="ExternalInput")  # x[512:1024], [s, i]
    btr = nc.dram_tensor("btr", [IN, N], f16, kind="ExternalInput")
    bti = nc.dram_tensor("bti", [IN, N], f16, kind="ExternalInput")
    vre = nc.dram_tensor("vre", [WCON, N], f16, kind="ExternalInput")
    vim = nc.dram_tensor("vim", [WCON, N], f16, kind="ExternalInput")
    bn2 = nc.dram_tensor("bn2", [N, 2 * IN], f16, kind="ExternalInput")
    bn3 = nc.dram_tensor("bn3", [N, 2 * IN], f16, kind="ExternalInput")
    cosj = nc.dram_tensor("cosj", [N, HF], f16, kind="ExternalInput")
    sinj = nc.dram_tensor("sinj", [N, HF], f16, kind="ExternalInput")
    rb = nc.dram_tensor("rb", [N, 1], f32, kind="ExternalInput")
    # rot columns: 0=cos(theta), 1=-sin(theta), 2=sin(theta)
    rot = nc.dram_tensor("rot", [N, 3], f32, kind="ExternalInput")
    ctr = nc.dram_tensor("ctr", [N, OUT], f16, kind="ExternalInput")
    ctin = nc.dram_tensor("ctin", [N, OUT], f16, kind="ExternalInput")
    dtw = nc.dram_tensor("dtw", [IN, OUT], f16, kind="ExternalInput")
    yT = nc.dram_tensor("yT", [OUT, DS], f16, kind="ExternalOutput")

    with TileContext(nc) as tc:
        with (
            tc.tile_pool(name="const", bufs=1) as cp,
            tc.tile_pool(name="work", bufs=1) as wkp,
            tc.tile_pool(name="ps", bufs=3, space="PSUM") as bp,
            tc.tile_pool(name="yps", bufs=5, space="PSUM") as yp,
        ):
            # ---------------- DMA loads (priority-chunked) ----------------
            def row_tiles(dram, eng, pfx, rows=None, col0=0, cols=None):
                """Load dram[rb*P:(rb+1)*P, col0:col0+cols] per row-block."""
                rows = rows if rows is not None else dram.shape[0]
                cols = cols if cols is not None else dram.shape[1]
                out = []
                for i in range(rows // P):
                    t = cp.tile([P, cols], dram.dtype, tag=f"{pfx}{i}", name=f"{pfx}{i}")
                    eng.dma_start(t[:], dram[i * P:(i + 1) * P, col0:col0 + cols])
                    out.append(t)
                return out

            # scalar queue (HWDGE, then free for evacs): input + W-gemm operands
            btr_t = row_tiles(btr, nc.scalar, "btr")
            bti_t = row_tiles(bti, nc.scalar, "bti")
            xw_t = row_tiles(xw, nc.scalar, "xw")
            vre_t = row_tiles(vre, nc.scalar, "vre")
            vim_t = row_tiles(vim, nc.scalar, "vim")
            # sync queue (HWDGE): x halves h0-first, trig tables, bn tables
            xts = []
            for i in range(IBN):
                t = cp.tile([P, HF], f16, tag=f"xt{i}", name=f"xt{i}")
                nc.sync.dma_start(t[:, 0:HH], xT[i * P:(i + 1) * P, 0:HH])
                xts.append(t)
            cos_t = [None] * NB
            sin_t = [None] * NB
            def load_trig(nb):
                c = cp.tile([P, HF], f16, tag=f"cos{nb}", name=f"cos{nb}")
                s = cp.tile([P, HF], f16, tag=f"sin{nb}", name=f"sin{nb}")
                nc.sync.dma_start(c[:], cosj[nb * P:(nb + 1) * P, :])
                nc.sync.dma_start(s[:], sinj[nb * P:(nb + 1) * P, :])
                cos_t[nb], sin_t[nb] = c, s
            load_trig(0)
            for i in range(IBN):
                nc.sync.dma_start(xts[i][:, HH:HF], xT[i * P:(i + 1) * P, HH:HF])
            load_trig(1)
            bn2_t = row_tiles(bn2, nc.sync, "bn2")
            load_trig(2)
            bn3_t = row_tiles(bn3, nc.sync, "bn3")
            load_trig(3)
            rb_t = row_tiles(rb, nc.sync, "rb")
            rot_t = row_tiles(rot, nc.sync, "rot")
            # gpsimd queue (SWDGE, otherwise idle): late-needed output weights
            dtw_t = row_tiles(dtw, nc.gpsimd, "dtw")
            ctr_t = row_tiles(ctr, nc.gpsimd, "ctr")
            ctin_t = row_tiles(ctin, nc.gpsimd, "ctin")

            # ---------------- PE phase 1: input + W GEMMs ----------------
            bus = {}    # (nb, plane) -> [P, HF] f16 Bu tile
            w2s = [None] * NB  # [P, 2*IN] f16 [w_re | w_im]

            def input_gemm(nb):
                nsl = slice(nb * P, (nb + 1) * P)
                for pi, wtiles in enumerate((btr_t, bti_t)):
                    bu = wkp.tile([P, HF], f16, tag=f"bu{pi}", bufs=2,
                                  name=f"bu{pi}_{nb}")
                    ps = [bp.tile([P, HH], f32, tag="ps", name=f"bups{h}")
                          for h in range(2)]
                    for ib in range(IBN):
                        for h in range(2):  # same stationary weight, 2 moving
                            nc.tensor.matmul(
                                ps[h][:], wtiles[ib][:, nsl],
                                xts[ib][:, h * HH:(h + 1) * HH],
                                start=(ib == 0), stop=(ib == IBN - 1))
                    for h in range(2):
                        nc.scalar.copy(bu[:, h * HH:(h + 1) * HH], ps[h][:])
                    bus[(nb, pi)] = bu

            def w_gemm(nb):
                nsl = slice(nb * P, (nb + 1) * P)
                w2 = wkp.tile([P, 2 * IN], f16, tag="w2", bufs=2, name=f"w2_{nb}")
                for pi, vt in enumerate((vre_t, vim_t)):
                    ps = bp.tile([P, IN], f32, tag="ps", name="wps")
                    for sb in range(WB):
                        nc.tensor.matmul(ps[:], vt[sb][:, nsl], xw_t[sb][:],
                                         start=(sb == 0), stop=(sb == WB - 1))
                    nc.scalar.copy(w2[:, pi * IN:(pi + 1) * IN], ps[:])
                w2s[nb] = w2

            # ---------------- DVE helpers ----------------
            dump = wkp.tile([P, 2 * IN], f16, tag="dump", name="dump")
            dump2 = wkp.tile([P, 2 * IN], f16, tag="dump2", name="dump2")
            carries = {}

            def carry_reduce(nb):
                """a = sum_i bn (.) w2 (row-dot), then init = e^{i theta}*a."""
                a_re = cp.tile([P, 1], f32, tag=f"are{nb}", name=f"are{nb}")
                a_im = cp.tile([P, 1], f32, tag=f"aim{nb}", name=f"aim{nb}")
                eng_im = nc.gpsimd if GP_STT else nc.vector
                nc.vector.scalar_tensor_tensor(
                    dump[:], bn2_t[nb][:], 1.0, w2s[nb][:], AOP.bypass, AOP.mult,
                    accum_out=a_re[:])
                eng_im.scalar_tensor_tensor(
                    dump2[:], bn3_t[nb][:], 1.0, w2s[nb][:], AOP.bypass, AOP.mult,
                    accum_out=a_im[:])
                carries[nb] = (a_re, a_im)

            inits = {}

            def carry_rot(nb):
                a_re, a_im = carries[nb]
                i_re = cp.tile([P, 1], f32, tag=f"ire{nb}", name=f"ire{nb}")
                i_im = cp.tile([P, 1], f32, tag=f"iim{nb}", name=f"iim{nb}")
                u_re = cp.tile([P, 1], f32, tag=f"ure{nb}", name=f"ure{nb}")
                u_im = cp.tile([P, 1], f32, tag=f"uim{nb}", name=f"uim{nb}")
                nc.scalar.mul(u_re[:], a_re[:], rot_t[nb][:, 0:1])
                nc.vector.scalar_tensor_tensor(
                    i_re[:], a_im[:], rot_t[nb][:, 1:2], u_re[:],
                    AOP.mult, AOP.add)
                nc.scalar.mul(u_im[:], a_im[:], rot_t[nb][:, 0:1])
                nc.vector.scalar_tensor_tensor(
                    i_im[:], a_re[:], rot_t[nb][:, 2:3], u_im[:],
                    AOP.mult, AOP.add)
                inits[nb] = (i_re, i_im)

            es = {}

            def twist(nb):
                ct, st = cos_t[nb], sin_t[nb]
                br, bi = bus[(nb, 0)], bus[(nb, 1)]
                p1 = wkp.tile([P, HF], f16, tag="p1", bufs=2, name="p1")
                p2 = wkp.tile([P, HF], f16, tag="p2", bufs=2, name="p2")
                e_re = wkp.tile([P, HF], f16, tag="er", bufs=4, name=f"er{nb}")
                e_im = wkp.tile([P, HF], f16, tag="ei", bufs=4, name=f"ei{nb}")
                nc.vector.tensor_tensor(p1[:], ct[:], br[:], AOP.mult)
                nc.vector.tensor_tensor(p2[:], st[:], bi[:], AOP.mult)
                nc.vector.tensor_tensor(e_re[:], p1[:], p2[:], AOP.add)
                nc.vector.tensor_tensor(p1[:], ct[:], bi[:], AOP.mult)
                nc.vector.tensor_tensor(p2[:], st[:], br[:], AOP.mult)
                nc.vector.tensor_tensor(e_im[:], p1[:], p2[:], AOP.subtract)
                es[nb] = (e_re, e_im)

            ss = {}

            def scan(nb, h):
                """Scan chunk h (FD=512 runs ~1.47cyc/elem vs 2.0 at FD=1024)."""
                i_re, i_im = inits[nb]
                e_re, e_im = es[nb]
                if h == 0:
                    s_re = wkp.tile([P, HF], f16, tag="sr", bufs=4, name=f"sr{nb}")
                    s_im = wkp.tile([P, HF], f16, tag="si", bufs=4, name=f"si{nb}")
                    ss[nb] = (s_re, s_im)
                s_re, s_im = ss[nb]
                hs = slice(h * HH, (h + 1) * HH)
                ir = i_re[:, 0:1] if h == 0 else s_re[:, HH - 1:HH]
                ii = i_im[:, 0:1] if h == 0 else s_im[:, HH - 1:HH]
                rbb = rb_t[nb][:, 0:1].broadcast_to((P, HH))
                nc.vector.tensor_tensor_scan(
                    s_re[:, hs], rbb, e_re[:, hs], ir, AOP.mult, AOP.add)
                nc.vector.tensor_tensor_scan(
                    s_im[:, hs], rbb, e_im[:, hs], ii, AOP.mult, AOP.add)

            hhs = {}

            def untwist(nb, h):
                hs = slice(h * HH, (h + 1) * HH)
                s_re, s_im = ss[nb]
                ct, st = cos_t[nb], sin_t[nb]
                if h == 0:
                    hhr = wkp.tile([P, HF], f16, tag="hhr", bufs=4, name=f"hhr{nb}")
                    hhi = wkp.tile([P, HF], f16, tag="hhi", bufs=4, name=f"hhi{nb}")
                    hhs[nb] = (hhr, hhi)
                hhr, hhi = hhs[nb]
                q1 = wkp.tile([P, HH], f16, tag="q1", bufs=2, name="q1")
                q2 = wkp.tile([P, HH], f16, tag="q2", bufs=2, name="q2")
                q3 = wkp.tile([P, HH], f16, tag="q3", bufs=2, name="q3")
                q4 = wkp.tile([P, HH], f16, tag="q4", bufs=2, name="q4")
                eng4 = nc.gpsimd if GP_UNTW else nc.vector
                eng4.tensor_tensor(q4[:], st[:, hs], s_re[:, hs], AOP.mult)
                nc.vector.tensor_tensor(q1[:], ct[:, hs], s_re[:, hs], AOP.mult)
                nc.vector.tensor_tensor(q2[:], st[:, hs], s_im[:, hs], AOP.mult)
                nc.vector.tensor_tensor(hhr[:, hs], q1[:], q2[:], AOP.subtract)
                nc.vector.tensor_tensor(q3[:], ct[:, hs], s_im[:, hs], AOP.mult)
                nc.vector.tensor_tensor(hhi[:, hs], q3[:], q4[:], AOP.add)

            # ---------------- emit phase 1 + DVE chain ----------------
            input_gemm(0)
            w_gemm(0)
            twist(0)
            carry_reduce(0)
            input_gemm(1)
            w_gemm(1)
            twist(1)
            carry_reduce(1)
            input_gemm(2)
            w_gemm(2)
            twist(2)
            carry_reduce(2)
            input_gemm(3)
            w_gemm(3)
            twist(3)
            carry_reduce(3)
            for nb in range(NB):
                carry_rot(nb)

            # ---------------- output GEMMs: D@x first (fills PE gap) -----
            NMM = 2 * NB + IBN  # accumulation ops per output group
            groups = {}
            for h in range(2):
                for ob in range(OBN):
                    gi = h * OBN + ob
                    pool = yp if gi < 5 else bp
                    tag = "yps" if gi < 5 else "ps"
                    groups[(h, ob)] = pool.tile([P, HH], f32, tag=tag, name="yps")
            # dtw matmuls: weight-stationary over (h in groups)
            for ob in range(OBN):
                osl = slice(ob * P, (ob + 1) * P)
                for ib in range(IBN):
                    for h in range(2):
                        nc.tensor.matmul(
                            groups[(h, ob)][:], dtw_t[ib][:, osl],
                            xts[ib][:, h * HH:(h + 1) * HH],
                            start=(ib == 0), stop=False)

            # ---------------- scans + untwists + C matmuls ----------------
            def store_group(h, ob):
                osl = slice(ob * P, (ob + 1) * P)
                hsl = slice(h * HH, (h + 1) * HH)
                ysb = wkp.tile([P, HH], f16, tag="ysb", bufs=4, name="ysb")
                nc.scalar.copy(ysb[:], groups[(h, ob)][:])
                eng = nc.sync if ob % 2 == 0 else nc.gpsimd
                eng.dma_start(yT[osl, hsl], ysb[:])

            def c_mms(nb):
                hhr, hhi = hhs[nb]
                last = nb == NB - 1
                for ob in range(OBN):
                    osl = slice(ob * P, (ob + 1) * P)
                    for wi, (wt, m) in enumerate(
                            ((ctr_t[nb][:, osl], hhr), (ctin_t[nb][:, osl], hhi))):
                        for h in range(2):
                            nc.tensor.matmul(
                                groups[(h, ob)][:], wt, m[:, h * HH:(h + 1) * HH],
                                start=False, stop=(last and wi == 1))
                    if last:
                        store_group(0, ob)
                        store_group(1, ob)

            for nb in range(NB):
                scan(nb, 0)
                untwist(nb, 0)
                scan(nb, 1)
                untwist(nb, 1)
                c_mms(nb)

    nc.compile()
    nc.finalize()
    _CACHE["nc"] = nc
    return nc


def _host_prep(x, nu_log, theta_log, gamma_log, B_re, B_im, C_re, C_im, D):
    f64 = np.float64
    nu = np.asarray(nu_log, f64)
    th = np.asarray(theta_log, f64)
    gl = np.asarray(gamma_log, f64)
    r = np.exp(-np.exp(nu))
    theta = np.exp(th)
    gamma = np.exp(gl)

    gbr = gamma[:, None] * np.asarray(B_re, f64)
    gbi = gamma[:, None] * np.asarray(B_im, f64)
    shared = {
        "btr": np.ascontiguousarray(gbr.T).astype(np.float16),
        "bti": np.ascontiguousarray(gbi.T).astype(np.float16),
        "ctr": np.ascontiguousarray(np.asarray(C_re, f64).T).astype(np.float16),
        "ctin": np.ascontiguousarray((-np.asarray(C_im, f64)).T).astype(np.float16),
        "dtw": np.ascontiguousarray(np.asarray(D, f64).T).astype(np.float16),
    }
    j = np.arange(HF, dtype=f64)
    ang = theta[:, None] * j[None, :]
    shared["cosj"] = np.cos(ang).astype(np.float16)
    shared["sinj"] = np.sin(ang).astype(np.float16)
    # V = lam^{511-s} over the LAST WCON steps of the first half, [s, n]
    e = (WCON - 1) - np.arange(WCON, dtype=f64)
    mag = np.exp(np.log(r)[:, None] * e[None, :])
    angv = theta[:, None] * e[None, :]
    shared["vre"] = np.ascontiguousarray((mag * np.cos(angv)).T).astype(np.float16)
    shared["vim"] = np.ascontiguousarray((mag * np.sin(angv)).T).astype(np.float16)
    shared["bn2"] = np.concatenate([gbr, -gbi], axis=1).astype(np.float16)
    shared["bn3"] = np.concatenate([gbi, gbr], axis=1).astype(np.float16)
    shared["rb"] = np.ascontiguousarray(r[:, None].astype(np.float32))
    shared["rot"] = np.stack(
        [np.cos(theta), -np.sin(theta), np.sin(theta)], axis=1).astype(np.float32)

    x = np.asarray(x, np.float32)
    in_maps = []
    for b in range(BATCH):
        m = dict(shared)
        m["xT"] = np.ascontiguousarray(x[b, HF:].T).astype(np.float16)
        m["xw"] = np.ascontiguousarray(x[b, HF - WCON:HF]).astype(np.float16)
        in_maps.append(m)
    return in_maps


def _run(in_maps, trace=False):
    nc = _build_nc()
    return run_bass_kernel_spmd(nc, in_maps, core_ids=list(range(BATCH)), trace=trace)


def kernel(**inputs):
    in_maps = _host_prep(**inputs)
    res = _run(in_maps, trace=False)
    y = np.stack([np.ascontiguousarray(res.results[b]["yT"].T) for b in range(BATCH)])
    return y.astype(np.float32)


def kernel_traced(**inputs):
    """Like kernel() but returns (y, exec_time_ns). Used by test.py."""
    in_maps = _host_prep(**inputs)
    res = _run(in_maps, trace=True)
    y = np.stack([np.ascontiguousarray(res.results[b]["yT"].T) for b in range(BATCH)])
    return y.astype(np.float32), res.exec_time_ns
